# revision 14
# baseline (speedup 1.0000x reference)
"""Trainium2 Bass kernel for the MCA (multi-axis pooled gating) module.

Computation (per sample b):
    hw_m = mean_{u,v} x   uv_m = mean_{h,w} x   uh_m = mean_{v,w} x   vw_m = mean_{u,h} x
    body = conv2(silu(conv1(uvhw)))   (1x1 convs on the packed (H+V, W+U) pooled map)
    gates: hw_g = f0(body_hw), uv_g = f1(body_uv), uh_g = f2(body_uh), vw_g = f3(body_vw)
    out = x * (hw_g + uv_g + uh_g + vw_g)      (each gate broadcast to the 6D shape)

Distribution: 8 cores = 4 samples x 2 h-halves. Each core owns
x[b, :, :, :, hh*32:(hh+1)*32, :], held in SBUF as bf16. The only cross-core
data are the h-reduced pools (uv_m, vw_m partials), exchanged as two pair
AllReduces (A: v0-2, B: v3-4) — SBUF-to-SBUF, fp32 (bf16 collectives measured
2.5x slower), split so group-A gating overlaps collective B. The partials
stay hs-unfolded across the exchange: the uv/vw conv chain contracts over all
128 partitions with hs-replicated w1 weights, folding the halves for free,
and the final conv uses column-doubled f1/f3 weights so the gates land on
all 128 partitions directly (no mirror DMA).

On-core layout: SBUF partition p = hs*64 + c, where the core's 32 h-rows
split as h2 = hs*16 + hl.

Engine budget: PE does the (u,v)-pool accumulation, the channel-mixing convs
and 15 of 25 phase-3 gate-broadcast adds (identity accumulating matmuls);
DVE does the w-pool fold trees, 10 gate adds and all final multiplies
(batched 2 per v); GpSimd does the phase-1 vw hl-folds and the small
post-collective scale/qbuf ops; ACT does PSUM evacuation and SiLU/biases.
A paced PE filler chain spans the collective wait so the HAM clock-gate
warm-up cost is paid before the gating burst, not during it.
"""

import sys
if '/opt/trn_rl_repo' not in sys.path:
    sys.path.insert(0, '/opt/trn_rl_repo')

from contextlib import ExitStack

import numpy as np
import concourse.bass as bass
import concourse.bacc as bacc
import concourse.tile as tile
from concourse import mybir

F32 = mybir.dt.float32
BF16 = mybir.dt.bfloat16
AF = mybir.ActivationFunctionType
ALU = mybir.AluOpType

# per-(v,u) G-add route. GpSimd streaming contends with DVE's SBUF ports, so
# phase 3 uses only PE (+ACT evac) and DVE.
V_ROUTE = ("pe", "pe", "dve", "pe", "dve")


def _ap(t_ap, dims, extra_off=0):
    """Manual free-dim view of an AP: dims = [(step_elems, count), ...]."""
    return bass.AP(
        tensor=t_ap.tensor,
        offset=t_ap.offset + extra_off,
        ap=[list(t_ap.ap[0])] + [[s, c] for (s, c) in dims],
    )


def build_program(C=64, U=5, V=5, H2=32, W=64, n_cores=8, sbuf_cc=True):
    """One SPMD program; per-core inputs select the (b, h-half) shard."""
    assert C == 64 and H2 % 2 == 0
    HL = H2 // 2              # h rows per hs partition group
    P = 2 * C                 # 128 partitions = (hs, c)
    CHW = HL * W              # free size of one (u,v) chunk per partition
    NMM = min(512, CHW)       # matmul moving-operand max (PSUM bank)
    NUV = U * V
    NB = U + W                # per-v partials block: [uv_u | vw_w]
    H = 2 * H2
    VA = 3                    # v-count in collective group A

    nc = bacc.Bacc('TRN2', target_bir_lowering=False, debug=False,
                   enable_asserts=False, num_devices=n_cores)

    x_d = nc.dram_tensor("x", [V, P, U, HL, W], BF16, kind="ExternalInput").ap()
    out_d = nc.dram_tensor("out", [V, P, U, HL, W], BF16,
                           kind="ExternalOutput").ap()
    # cpack columns: [ident(P) | w1T,w2T,f0T,f2T (C each) | f1T2,f3T2 (2C each)]
    NCON = P + 4 * C + 2 * 2 * C
    cpack_d = nc.dram_tensor("cpack", [P, NCON], BF16, kind="ExternalInput").ap()
    cbias_d = nc.dram_tensor("cbias", [P, 8], F32, kind="ExternalInput").ap()

    groups = [[2 * i, 2 * i + 1] for i in range(n_cores // 2)]

    with tile.TileContext(nc) as tc, ExitStack() as ctx:
        consts = ctx.enter_context(tc.tile_pool(name="consts", bufs=1))
        xpool = ctx.enter_context(tc.tile_pool(name="x", bufs=V))
        sumu_pool = ctx.enter_context(tc.tile_pool(name="sumu", bufs=2))
        small = ctx.enter_context(tc.tile_pool(name="small", bufs=1))
        convp = ctx.enter_context(tc.tile_pool(name="convp", bufs=2))
        ppool = ctx.enter_context(tc.tile_pool(name="pp", bufs=U))
        gpool = ctx.enter_context(tc.tile_pool(name="gpool", bufs=2))
        opool = ctx.enter_context(tc.tile_pool(name="opool", bufs=2))
        phase1_ctx = ExitStack()
        ps_acc = phase1_ctx.enter_context(
            tc.tile_pool(name="ps_acc", bufs=3, space="PSUM"))
        ps_hw = phase1_ctx.enter_context(
            tc.tile_pool(name="ps_hw", bufs=1, space="PSUM"))
        dram = ctx.enter_context(tc.tile_pool(name="dram", bufs=1, space="DRAM"))

        # x loads first: they gate everything
        xv_t = []
        for v in range(V):
            xv = xpool.tile([P, U, HL, W], BF16, tag="xv", name=f"xv{v}")
            xv_t.append(xv)
            nc.sync.dma_start(out=xv[:], in_=x_d[v])

        cpack = consts.tile([P, NCON], BF16)
        nc.gpsimd.dma_start(out=cpack[:], in_=cpack_d[:, :])
        cbias = consts.tile([P, 8], F32)
        nc.gpsimd.dma_start(out=cbias[:], in_=cbias_d[:, :])
        id16 = cpack[:, 0:P]
        wnames = ("w1T", "w2T", "f0T", "f2T")
        wt = {nm: cpack[:, P + i * C:P + (i + 1) * C]
              for i, nm in enumerate(wnames)}
        w2names = ("f1T2", "f3T2")
        for i, nm in enumerate(w2names):
            base = P + 4 * C + i * 2 * C
            wt[nm] = cpack[:, base:base + 2 * C]
        bnames = ("b1", "b2", "fb0", "fb1", "fb2", "fb3")
        bt = {nm: cbias[0:C, i:i + 1] for i, nm in enumerate(bnames)}
        btP = {nm: cbias[0:P, i:i + 1] for i, nm in enumerate(bnames)}

        def mm16(out_ps, rhs, start, stop):
            nc.tensor.matmul(out_ps, id16, rhs, start=start, stop=stop)

        def mmw(out_ps, lhsT, rhs, start=True, stop=True):
            nc.tensor.matmul(out_ps, lhsT, rhs, start=start, stop=stop)

        # ---------------- Phase 1: pools -------------------------
        partials = small.tile([P, V * NB], F32)   # per-v blocks [uv_u | vw_w]
        s_w = small.tile([P, V, U, HL], F32)      # x summed over w
        hw_ps = ps_hw.tile([P, CHW], F32)         # x summed over (u, v)
        cc_out = {}

        def emit_cc(g, v0, v1):
            n = (v1 - v0) * NB
            cid = dram.tile([P, n], F32, name=f"ccin_{g}", tag=f"ccin_{g}")
            cod = dram.tile([P, n], F32, name=f"ccout_{g}", tag=f"ccout_{g}")
            nc.sync.dma_start(out=cid[:], in_=partials[:, v0 * NB:v1 * NB])
            nc.gpsimd.collective_compute(
                "AllReduce", ALU.add, replica_groups=groups,
                ins=[cid[:].opt()], outs=[cod[:].opt()])
            co = small.tile([P, n], F32, name=f"cc_{g}")
            nc.scalar.dma_start(out=co[:], in_=cod[:])
            cc_out[g] = co

        for v in range(V):
            xv = xv_t[v]
            acc = ps_acc.tile([P, CHW], F32, tag="acc")   # sum over u, this v
            for u in range(U):
                for j0 in range(0, CHW, NMM):
                    mm16(acc[:, j0:j0 + NMM],
                         xv[:, u].rearrange("p hl w -> p (hl w)")[:, j0:j0 + NMM],
                         start=(u == 0), stop=(u == U - 1))
            # evacuate acc to SBUF bf16 (feeds hw accumulation + vw hl-fold)
            sumu = sumu_pool.tile([P, CHW], BF16, tag="sumu")
            nc.scalar.copy(out=sumu[:], in_=acc[:])
            # hw accumulation back through the PE
            for j0 in range(0, CHW, NMM):
                mm16(hw_ps[:, j0:j0 + NMM], sumu[:, j0:j0 + NMM],
                     start=(v == 0), stop=(v == V - 1))
            # vw partial: fold hl out
            if v < V - 1:
                # GpSimd halving tree over sumu (keeps DVE free)
                t1 = sumu_pool.tile([P, (HL // 2) * W], BF16, tag="vt1")
                nc.gpsimd.tensor_add(t1[:], sumu[:, 0:(HL // 2) * W],
                                     sumu[:, (HL // 2) * W:CHW])
                t2 = sumu_pool.tile([P, (HL // 4) * W], BF16, tag="vt2")
                nc.gpsimd.tensor_add(t2[:], t1[:, 0:(HL // 4) * W],
                                     t1[:, (HL // 4) * W:])
                t3 = sumu_pool.tile([P, (HL // 8) * W], BF16, tag="vt3")
                nc.gpsimd.tensor_add(t3[:], t2[:, 0:(HL // 8) * W],
                                     t2[:, (HL // 8) * W:])
                nc.gpsimd.tensor_add(partials[:, v * NB + U:(v + 1) * NB],
                                     t3[:, 0:W], t3[:, W:2 * W])
            else:
                # last v: reduce straight from PSUM on DVE — skips the
                # ACT-evac + GpSimd chain so collective B triggers sooner
                nc.vector.tensor_reduce(
                    partials[:, v * NB + U:(v + 1) * NB],
                    _ap(acc[:], [(1, W), (W, HL)]),
                    axis=mybir.AxisListType.X, op=ALU.add)
            # s_w (sum over w): two bf16 pair-folds (DVE 2x perf mode) then a
            # reduce over the remaining 16 columns
            f1 = sumu_pool.tile([P, U, HL, W // 2], BF16, tag="swf1")
            nc.vector.tensor_add(
                f1[:], _ap(xv[:], [(HL * W, U), (W, HL), (1, W // 2)]),
                _ap(xv[:], [(HL * W, U), (W, HL), (1, W // 2)],
                    extra_off=W // 2))
            f2 = sumu_pool.tile([P, U, HL, W // 4], BF16, tag="swf2")
            nc.vector.tensor_add(
                f2[:], _ap(f1[:], [(HL * W // 2, U), (W // 2, HL), (1, W // 4)]),
                _ap(f1[:], [(HL * W // 2, U), (W // 2, HL), (1, W // 4)],
                    extra_off=W // 4))
            nc.vector.tensor_reduce(s_w[:, v], f2[:],
                                    axis=mybir.AxisListType.X, op=ALU.add)
            # uv partial for this v
            nc.vector.tensor_reduce(partials[:, v * NB:v * NB + U], s_w[:, v],
                                    axis=mybir.AxisListType.X, op=ALU.add)
            if v == VA - 1:
                emit_cc("A", 0, VA)
            elif v == V - 1:
                emit_cc("B", VA, V)

        # uh local sums -> means
        uh_raw = small.tile([P, U, HL], F32)
        swv = s_w[:].rearrange("p v u hl -> p u hl v")
        nc.vector.tensor_reduce(uh_raw[:], swv, axis=mybir.AxisListType.X,
                                op=ALU.add)
        uh_sc = small.tile([P, U * HL], BF16)
        nc.scalar.activation(out=uh_sc[:],
                             in_=uh_raw[:].rearrange("p u hl -> p (u hl)"),
                             func=AF.Copy, scale=1.0 / (V * W))
        # hw means
        hw_m = small.tile([P, CHW], BF16)
        nc.scalar.activation(out=hw_m[:], in_=hw_ps[:],
                             func=AF.Copy, scale=1.0 / NUV)
        phase1_ctx.close()   # release pool-phase PSUM banks
        ps1p = ctx.enter_context(tc.tile_pool(name="ps1p", bufs=2, space="PSUM"))
        ps2p = ctx.enter_context(tc.tile_pool(name="ps2p", bufs=1, space="PSUM"))
        ps3p = ctx.enter_context(tc.tile_pool(name="ps3p", bufs=1, space="PSUM"))
        psgp = ctx.enter_context(tc.tile_pool(name="psg", bufs=2, space="PSUM"))

        # gate buffers (same pixel orders as the conv inputs)
        hwg = small.tile([P, CHW], BF16)         # (hl, w) per (hs,c) partition
        uhg = small.tile([P, U * HL], BF16)      # (u, hl) per (hs,c) partition
        gates = {}                               # per-group vw/uv gate tiles

        def run_conv_jobs(jobs):
            """Software-pipelined 1x1-conv chains (2 jobs in flight).

            Each job: (rhs_ap, nn, hs, f_nm, fb_nm, target). hs in (0, 1):
            per-hs job, rhs [C, nn] on that hs's partitions. hs == 'fullP':
            rhs [P, nn] hs-unfolded — the w1 contraction over 128 partitions
            folds the halves; column-doubled final weights put the gate on
            all 128 partitions.
            """
            ps1s = [None] * len(jobs)
            for j in range(len(jobs) + 2):
                if j < len(jobs):
                    rhs, nn, hs, f_nm, fb_nm, target = jobs[j]
                    if hs == 'fullP':
                        w1 = wt["w1T"][0:P, :]
                    else:
                        w1 = wt["w1T"][hs * C:(hs + 1) * C, :]
                    ps1 = ps1p.tile([C, nn], F32, tag="ps1")
                    mmw(ps1[:], w1, rhs)
                    ps1s[j] = ps1
                k = j - 2
                if k < 0 or k >= len(jobs):
                    continue
                rhs, nn, hs, f_nm, fb_nm, target = jobs[k]
                ps1 = ps1s[k]
                sig = convp.tile([C, nn], BF16, tag="sig")
                nc.scalar.activation(out=sig[:], in_=ps1[:], func=AF.Sigmoid,
                                     bias=bt["b1"])
                a1 = convp.tile([C, nn], BF16, tag="a1")
                nc.vector.scalar_tensor_tensor(
                    out=a1[:], in0=ps1[:], scalar=bt["b1"],
                    in1=sig[:], op0=ALU.add, op1=ALU.mult)
                ps2 = ps2p.tile([C, nn], F32, tag="ps2")
                mmw(ps2[:], wt["w2T"][0:C, :], a1[:])
                body = convp.tile([C, nn], BF16, tag="body")
                nc.vector.tensor_scalar(out=body[:], in0=ps2[:],
                                        scalar1=bt["b2"], scalar2=None,
                                        op0=ALU.add)
                if hs == 'fullP':
                    ps3 = ps3p.tile([P, nn], F32, tag="ps3")
                    mmw(ps3[:], wt[f_nm][0:C, :], body[:])
                    nc.scalar.activation(out=target, in_=ps3[:],
                                         func=AF.Identity, bias=btP[fb_nm])
                else:
                    ps3 = ps3p.tile([C, nn], F32, tag="ps3")
                    mmw(ps3[:], wt[f_nm][0:C, :], body[:])
                    if hs == 0:
                        nc.scalar.activation(out=target, in_=ps3[:],
                                             func=AF.Identity, bias=bt[fb_nm])
                    else:
                        gate = convp.tile([C, nn], BF16, tag="gate")
                        nc.scalar.activation(out=gate[:], in_=ps3[:],
                                             func=AF.Identity, bias=bt[fb_nm])
                        nc.sync.dma_start(out=target, in_=gate[:])

        # local jobs: hw (per hs, per 512-chunk) and uh (per hs)
        jobs = []
        for hs in range(2):
            for j0 in range(0, CHW, NMM):
                jobs.append((hw_m[hs * C:(hs + 1) * C, j0:j0 + NMM], NMM, hs,
                             "f0T", "fb0",
                             hwg[hs * C:(hs + 1) * C, j0:j0 + NMM]))
        for hs in range(2):
            jobs.append((uh_sc[hs * C:(hs + 1) * C], U * HL, hs,
                         "f2T", "fb2", uhg[hs * C:(hs + 1) * C, :]))
        run_conv_jobs(jobs)

        # P_u = hwg + uhg[:, u, :] broadcast over w (local, pre-collective)
        pbufs = []
        for u in range(U):
            pbuf = ppool.tile([P, CHW], BF16, tag="p")
            uh_b = _ap(uhg[:], [(1, HL), (0, W)], extra_off=u * HL)
            nc.vector.tensor_add(pbuf[:], hwg[:], uh_b)
            pbufs.append(pbuf)

        # PE warmth filler: a paced MM->ACT->MM chain spanning the collective
        # wait so the HAM clock-gate warm-up is paid before the gating burst.
        # Results are garbage and never read downstream.
        scr = small.tile([P, NMM], BF16, name="warm_scr")
        warm_src = hw_m[:, 0:NMM]
        for k in range(20):
            psw = ps1p.tile([P, NMM], F32, tag="ps1")
            mm16(psw[:], warm_src, start=True, stop=False)
            mm16(psw[:], hw_m[:, 0:NMM], start=False, stop=True)
            nc.scalar.copy(out=scr[:], in_=psw[:])
            warm_src = scr[:]

        qbuf = small.tile([P, V, U, W], BF16)

        def post_group(g, v0, v1):
            """Scale + uv/vw convs + per-v qbuf for one collective group."""
            co = cc_out[g]
            cnt = v1 - v0
            nuv = cnt * U
            pad = nuv % 2
            uv_sc = small.tile([P, nuv + pad], BF16, name=f"uvsc_{g}")
            vw_sc = small.tile([P, cnt * W], BF16, name=f"vwsc_{g}")
            uv_src = _ap(co[:], [(NB, cnt), (1, U)])
            nc.gpsimd.tensor_scalar(
                out=uv_sc[:, 0:nuv], in0=uv_src,
                scalar1=1.0 / (H * W), scalar2=None, op0=ALU.mult)
            if pad:
                nc.gpsimd.tensor_scalar(
                    out=uv_sc[:, nuv:nuv + 1], in0=co[:, 0:1],
                    scalar1=1.0, scalar2=None, op0=ALU.mult)
            vw_src = _ap(co[:], [(NB, cnt), (1, W)], extra_off=U)
            nc.gpsimd.tensor_scalar(
                out=vw_sc[:, :], in0=vw_src,
                scalar1=1.0 / (U * H), scalar2=None, op0=ALU.mult)
            vwg = small.tile([P, cnt * W], BF16, name=f"vwg_{g}")
            uvg = small.tile([P, nuv + pad], BF16, name=f"uvg_{g}")
            gates[g] = (vwg, uvg)
            run_conv_jobs([
                (vw_sc[:, :], cnt * W, 'fullP', "f3T2", "fb3", vwg[:, :]),
                (uv_sc[:, :], nuv + pad, 'fullP', "f1T2", "fb1", uvg[:, :]),
            ])
            for v in range(v0, v1):
                vw_b = _ap(vwg[:], [(0, 1), (0, U), (1, W)],
                           extra_off=(v - v0) * W)
                uv_b = _ap(uvg[:], [(0, 1), (1, U), (0, W)],
                           extra_off=(v - v0) * U)
                nc.gpsimd.tensor_add(qbuf[:, v:v + 1], vw_b, uv_b)

        def gate_group(v0, v1):
            for v in range(v0, v1):
                gv = gpool.tile([P, U, CHW], BF16, tag="gv")
                obuf = opool.tile([P, U, HL, W], BF16, tag="obuf")
                for u in range(U):
                    q_off = (v * U + u) * W
                    if V_ROUTE[u] == "pe":
                        ps = psgp.tile([P, CHW], F32, tag="gps")
                        for hi in range(CHW // NMM):
                            sl = slice(hi * NMM, (hi + 1) * NMM)
                            mm16(ps[:, sl], pbufs[u][:, sl],
                                 start=True, stop=False)
                            nc.tensor.matmul(
                                ps[:, sl], id16,
                                _ap(qbuf[:], [(0, NMM // W), (1, W)],
                                    extra_off=q_off),
                                start=False, stop=True)
                        nc.scalar.copy(
                            out=gv[:, u].rearrange("p n -> p (n)"), in_=ps[:])
                    else:
                        q_b = _ap(qbuf[:], [(0, HL), (1, W)], extra_off=q_off)
                        nc.vector.tensor_add(
                            gv[:, u].rearrange("p n -> p (n)"),
                            pbufs[u][:], q_b)
                # two batched multiplies (u0-1, u2-4), each DMA'd out as it
                # completes so the store tail overlaps the remaining compute
                for (ua, ub) in ((0, 2), (2, 5)):
                    xin = xv_t[v][:, ua:ub].rearrange("p u hl w -> p (u hl w)")
                    oin = obuf[:, ua:ub].rearrange("p u hl w -> p (u hl w)")
                    gin = gv[:, ua:ub].rearrange("p u n -> p (u n)")
                    nc.vector.tensor_mul(oin, xin, gin)
                    nc.sync.dma_start(out=out_d[v][:, ua:ub],
                                      in_=obuf[:, ua:ub])

        post_group("A", 0, VA)
        gate_group(0, VA)
        post_group("B", VA, V)
        gate_group(VA, V)

    nc.compile()
    return nc


# ---------------------------------------------------------------------------
# Host entry point (full problem size, 8 cores)

B, C, U, V, H, W = 4, 64, 5, 5, 64, 64
H2 = H // 2
HL = H2 // 2

_prog_cache = {}


def _get_prog():
    if "nc" not in _prog_cache:
        _prog_cache["nc"] = build_program(C=C, U=U, V=V, H2=H2, W=W, n_cores=8)
    return _prog_cache["nc"]


def make_const_pack(inputs):
    import ml_dtypes
    P = 2 * C
    w1T = np.asarray(inputs["w1"], np.float32).T
    w2T = np.asarray(inputs["w2"], np.float32).T
    f = [np.asarray(inputs[f"fw{i}"], np.float32).T for i in range(4)]
    bs = [np.asarray(inputs["b1"], np.float32),
          np.asarray(inputs["b2"], np.float32)] + \
         [np.asarray(inputs[f"fb{i}"], np.float32) for i in range(4)]
    # columns: [ident(P) | w1T,w2T,f0T,f2T (C, hs-replicated rows) |
    #           f1T2,f3T2 (2C cols, top C rows only)]
    ncon = P + 4 * C + 2 * 2 * C
    cpack = np.zeros((P, ncon), dtype=np.float32)
    cpack[:, 0:P] = np.eye(P, dtype=np.float32)
    for i, w in enumerate((w1T, w2T, f[0], f[2])):
        cpack[:, P + i * C:P + (i + 1) * C] = np.vstack([w, w])
    for i, w in enumerate((f[1], f[3])):
        base = P + 4 * C + i * 2 * C
        cpack[0:C, base:base + 2 * C] = np.hstack([w, w])
    cbias = np.zeros((P, 8), dtype=np.float32)
    for i, b in enumerate(bs):
        cbias[0:C, i] = b
        cbias[C:2 * C, i] = b
    return cpack.astype(ml_dtypes.bfloat16), cbias


def make_in_maps(inputs):
    import ml_dtypes
    x = np.asarray(inputs["x"], dtype=np.float32)
    cpack, cbias = make_const_pack(inputs)
    base = {"cpack": cpack, "cbias": cbias}

    in_maps = []
    for core in range(8):
        b, hh = core // 2, core % 2
        s6 = x[b, :, :, :, hh * H2:(hh + 1) * H2, :]
        arr = np.ascontiguousarray(
            s6.reshape(C, U, V, 2, HL, W).transpose(2, 3, 0, 1, 4, 5))
        arr = arr.reshape(V, 2 * C, U, HL, W).astype(ml_dtypes.bfloat16)
        in_maps.append({"x": arr, **base})
    return in_maps


def assemble_out(results):
    out = np.empty((B, C, U, V, H, W), dtype=np.float32)
    for core in range(8):
        b, hh = core // 2, core % 2
        r = np.asarray(results[core]["out"]).astype(np.float32)
        r = r.reshape(V, 2, C, U, HL, W)
        out[b, :, :, :, hh * H2:(hh + 1) * H2, :] = (
            r.transpose(2, 3, 0, 1, 4, 5).reshape(C, U, V, H2, W))
    return out


def kernel(**inputs):
    from concourse.bass_utils import run_bass_kernel_spmd

    in_maps = make_in_maps(inputs)
    nc = _get_prog()
    res = run_bass_kernel_spmd(nc, in_maps, core_ids=list(range(8)))
    return assemble_out(res.results)
